# revision 22
# baseline (speedup 1.0000x reference)
"""Trainium2 Bass kernel for nn_Block2DGRU (norm->dwconv3x3->bi-minGRU->norm->MLP).

fp8(e4m3)+DoubleRow matmuls for conv/GRU/MLP (weights pre-scaled x32, conv x8,
unscaled via activation `scale=`); f32r (bitcast) for LN stats/broadcasts;
bf16 gate math on DVE (2x mode); the `h+0.5` pass on the Pool engine; conv
uses zero-padded image rows + SBUF->SBUF DMA column shifts, with the 9 taps
packed as 4 DoubleRow pairs + 1 single matmul.  SPMD over 8 NeuronCores,
2 batch elements per core.

Device layout: [feature_on_partitions, time_on_free].  minGRU recurrence
h_t = a_t*h_{t-1} + b_t runs on the DVE via tensor_tensor_scan; dir-2 is the
same scan with reversed access patterns.
"""
import numpy as np
import ml_dtypes

import concourse.bass as bass
import concourse.tile as tile
import concourse.mybir as mybir
from concourse.bass_utils import run_bass_kernel_spmd

F32 = mybir.dt.float32
F32R = mybir.dt.float32r
BF16 = mybir.dt.bfloat16
F8 = mybir.dt.float8e4
AF = mybir.ActivationFunctionType
ALU = mybir.AluOpType
DRM = mybir.MatmulPerfMode.DoubleRow

# dims
NB = 56
L = NB * NB            # 3136
D = 384                # dim
DC = 3                 # dim chunks of 128
DI = 768               # gru inner
MLPC = 12
B = 2                  # batch per core
NCORES = 8
NT = 392               # time block (= 7 image rows)
NBLK = L // NT         # 8
QT = 784               # scan quarter (= 2 blocks)
NQ = L // QT           # 4
EPS = 1e-5
PADL = 58 * NB         # image rows -1..56 -> 3248
SW = 32.0              # prescale for gru/mlp weights
SC = 8.0               # prescale for conv taps

MM_DT = F32R           # kept for test.py compat


def _pair_ap(flat_ap, off, stride, nt, nb=NB):
    """[128, 2, nt] read AP over a flat [128, X] AP: subtiles at element
    offsets `off` and `off+stride` (stride % nb == 0), each nt wide."""
    a3 = flat_ap.rearrange("p (r c) -> p r c", c=nb)
    s = stride // nb
    r0 = off // nb
    sl = a3[:, r0:r0 + s + 1:s, :]
    v = sl.ap
    v[2] = (1, nt)
    out = sl.copy()
    out.ap = v
    return out


# ---------------------------------------------------------------- wait fix
def _fix_multiwaits(nc):
    """The walrus accepts at most ONE sync wait per instruction; hoist
    extras into wait-only NoOps on the same engine (streams are in-order)."""
    n = 0
    cnt = [0]
    for f in nc.m.functions:
        for bb in f.blocks:
            out = []
            for inst in bb.instructions:
                si = inst.sync_info
                if si is not None and si.on_wait is not None and len(si.on_wait) > 1:
                    waits = list(si.on_wait)
                    for w in waits[:-1]:
                        cnt[0] += 1
                        nop = mybir.InstNoOp(
                            name=f"I-waitfix-{cnt[0]}",
                            sync_info=mybir.SyncInfo(on_wait=[w], on_update=[]),
                        )
                        nop.engine = inst.engine
                        out.append(nop)
                    inst.sync_info = mybir.SyncInfo(
                        on_wait=[waits[-1]], on_update=list(si.on_update or [])
                    )
                    n += 1
                out.append(inst)
            bb.instructions = out
    return n


# ---------------------------------------------------------------- builder
def build_kernel(reps=1):
    nc = bass.Bass("TRN2", target_bir_lowering=False, debug=False,
                   num_devices=NCORES)

    xT_d = nc.dram_tensor("xT", [B, D, L], F32, kind="ExternalInput").ap()
    whg_d = [nc.dram_tensor(f"whg{g}", [128, 6144], F8,
                            kind="ExternalInput").ap() for g in range(2)]
    wout_d = [nc.dram_tensor(f"wout{g}", [128, 2304], BF16,
                             kind="ExternalInput").ap() for g in range(2)]
    p1_d = nc.dram_tensor("p1", [128, 6144], F8, kind="ExternalInput").ap()
    p2_d = nc.dram_tensor("p2", [128, 4608], F8, kind="ExternalInput").ap()
    diag_d = nc.dram_tensor("diag", [DC, 128, 1152], F8,
                            kind="ExternalInput").ap()
    dwb_d = nc.dram_tensor("dwb", [128, DC], F32, kind="ExternalInput").ap()
    p1b_d = nc.dram_tensor("p1b", [128, MLPC], F32, kind="ExternalInput").ap()
    p2b_d = nc.dram_tensor("p2b", [128, DC], F32, kind="ExternalInput").ap()
    out_d = nc.dram_tensor("outT", [B, D, L], F32, kind="ExternalOutput").ap()

    from contextlib import ExitStack
    with tile.TileContext(nc) as tc, ExitStack() as ctx:
        big = ctx.enter_context(tc.tile_pool(name="big", bufs=1))
        wpool = ctx.enter_context(tc.tile_pool(name="wpool", bufs=1))
        work = ctx.enter_context(tc.tile_pool(name="work", bufs=2))
        psum = ctx.enter_context(tc.tile_pool(name="psum", bufs=2,
                                              space="PSUM"))

        # ---- persistent constants / weights
        ones_col = wpool.tile([128, 1], F32, tag="ones_col", name="ones_col")
        nc.vector.memset(ones_col[:], 1.0)
        ones_colr = wpool.tile([128, 1], F32R, tag="ones_colr",
                               name="ones_colr")
        nc.scalar.activation(ones_colr[:], ones_col[:], AF.Copy)
        ones1 = wpool.tile([1, 128], BF16, tag="ones1", name="ones1")
        nc.vector.memset(ones1[:], 1.0)
        ones_colb = wpool.tile([128, 1], BF16, tag="ones_colb",
                               name="ones_colb")
        nc.vector.memset(ones_colb[:], 1.0)
        dwb_t = wpool.tile([128, DC], F32, tag="dwb", name="dwb")
        nc.sync.dma_start(dwb_t[:], dwb_d)
        p1b_t = wpool.tile([128, MLPC], F32, tag="p1b", name="p1b")
        nc.sync.dma_start(p1b_t[:], p1b_d)
        p2b_t = wpool.tile([128, DC], F32, tag="p2b", name="p2b")
        nc.sync.dma_start(p2b_t[:], p2b_d)
        eps_t = wpool.tile([1, 1], F32, tag="eps", name="eps")
        nc.vector.memset(eps_t[:], EPS)
        neghalf = wpool.tile([128, 1], F32, tag="neghalf", name="neghalf")
        nc.vector.memset(neghalf[:], -0.5)

        whg_t, wout_t = [], []
        for g in range(2):
            t = wpool.tile([128, 6144], F8, tag=f"whg{g}", name=f"whg{g}")
            nc.sync.dma_start(t[:], whg_d[g])
            whg_t.append(t)
            t = wpool.tile([128, 2304], BF16, tag=f"wout{g}", name=f"wout{g}")
            nc.sync.dma_start(t[:], wout_d[g])
            wout_t.append(t)
        p1_t = wpool.tile([128, 6144], F8, tag="p1w", name="p1w")
        nc.sync.dma_start(p1_t[:], p1_d)
        p2_t = wpool.tile([128, 4608], F8, tag="p2w", name="p2w")
        nc.sync.dma_start(p2_t[:], p2_d)
        diag_t = []
        for c in range(DC):
            t = wpool.tile([128, 1152], F8, tag=f"diag{c}", name=f"diag{c}")
            nc.sync.dma_start(t[:], diag_d[c])
            diag_t.append(t)

        # ---- persistent big buffers
        xh_t = [big.tile([128, PADL], F8, tag=f"xh{c}", name=f"xh{c}")
                for c in range(DC)]
        xs_t = [big.tile([128, 2 * PADL], F8, tag=f"xs{c}", name=f"xs{c}")
                for c in range(DC)]
        hc01 = big.tile([128, 2 * L], F8, tag="hc01", name="hc01")
        hc2o = big.tile([128, 2 * L], F8, tag="hc2o", name="hc2o")
        y_t = [big.tile([128, L], F32, tag=f"y{c}", name=f"y{c}")
               for c in range(DC)]
        yh01 = big.tile([128, 2 * L], F8, tag="yh01", name="yh01")
        yh2o = big.tile([128, 2 * L], F8, tag="yh2o", name="yh2o")
        carry = [big.tile([128, 6], F32, tag=f"carry{g}", name=f"carry{g}")
                 for g in range(2)]

        # second halves of hc2o/yh2o hold a constant ones channel-block
        nc.gpsimd.memset(hc2o[:, L:2 * L], 1.0)
        nc.gpsimd.memset(yh2o[:, L:2 * L], 1.0)

        # one-time zeroing: pad rows (image rows -1, 56) + shift edge cols
        for c in range(DC):
            nc.gpsimd.memset(xh_t[c][:, 0:NB], 0.0)
            nc.gpsimd.memset(xh_t[c][:, PADL - NB:PADL], 0.0)
            xm3 = xs_t[c][:, 0:PADL].rearrange("p (r cc) -> p r cc", cc=NB)
            nc.gpsimd.memset(xm3[:, :, 0:1], 0.0)
            xp3 = xs_t[c][:, PADL:2 * PADL].rearrange("p (r cc) -> p r cc",
                                                      cc=NB)
            nc.gpsimd.memset(xp3[:, :, NB - 1:NB], 0.0)

        def hp_tile(name):
            return psum.tile([128, 1024], F32, tag="hp", name=name)

        def gp_tile(name):
            return psum.tile([128, 1024], F32, tag="gp", name=name)

        def pair2(t):
            """[128,1024] psum tile -> [128,2,NT] view (cols 0.., 512..)."""
            return t[:].rearrange("p (s h) -> p s h", s=2)[:, :, 0:NT]

        # ---------------------------------------------------- layernorm
        def norm(src_of, dst_of, order, inplace_src, src_is_r):
            """LN over chunks; src_of(c, t0) -> [128,NT] AP (dtype F32R
            when src_is_r else F32); dst_of(c, t0) -> fp8 dst AP.
            inplace_src: srcs may be clobbered by the apply."""
            for blk in order:
                t0 = blk * NT
                srcs = [src_of(c, t0) for c in range(DC)]
                srcf = [(sc.bitcast(F32) if src_is_r else sc) for sc in srcs]
                rows = work.tile([1, 2 * NT], BF16, tag="rows", name="rows",
                                 bufs=2)
                sq = [work.tile([128, NT], BF16, tag=f"sq{c}", name=f"sq{c}",
                                bufs=1) for c in range(DC)]
                sp = hp_tile("stat_ps")
                s_ps = sp[0:1, 0:NT]
                q_ps = sp[0:1, 512:512 + NT]
                for c in range(DC):
                    nc.gpsimd.tensor_tensor(sq[c][:], srcf[c], srcf[c],
                                            ALU.mult)
                for c in range(DC):
                    # f32r when the producer is a DMA (rounding-exempt);
                    # plain f32 (4 cyc) when the producer is the DVE
                    if src_is_r:
                        nc.tensor.matmul(s_ps, ones_colr[:], srcs[c],
                                         start=(c == 0), stop=(c == DC - 1))
                    else:
                        nc.tensor.matmul(s_ps, ones_col[:], srcs[c],
                                         start=(c == 0), stop=(c == DC - 1))
                for c in range(DC):
                    nc.tensor.matmul(q_ps, ones_colb[:], sq[c][:],
                                     start=(c == 0), stop=(c == DC - 1))
                numu = rows[0:1, 0:NT]
                inv = rows[0:1, NT:2 * NT]
                nc.scalar.activation(numu, s_ps, AF.Copy, scale=-1.0 / D)
                sf = work.tile([1, 4 * NT], F32, tag="statf", name="statf",
                               bufs=2)
                m2 = sf[0:1, 0:NT]
                ve = sf[0:1, NT:2 * NT]
                sd = sf[0:1, 2 * NT:3 * NT]
                nc.scalar.activation(m2, s_ps, AF.Square, scale=1.0 / D)
                nc.vector.scalar_tensor_tensor(ve, q_ps, 1.0 / D, m2,
                                               ALU.mult, ALU.subtract)
                # inv = 1/sqrt(ve+eps) = exp(-0.5*ln(ve+eps)); table ops
                # stay on Act (DVE reciprocal is a ~2.6us multi-pass op)
                nc.scalar.activation(sd, ve, AF.Ln, bias=eps_t[:])
                nc.scalar.activation(inv, sd, AF.Exp, scale=-0.5)
                # broadcast -mu / inv to 128 partitions (K=1 bf16 matmuls)
                bc = gp_tile("bc_ps")
                mb = bc[:, 0:NT]
                ib = bc[:, 512:512 + NT]
                nc.tensor.matmul(mb, ones1[:], numu, start=True, stop=True)
                nc.tensor.matmul(ib, ones1[:], inv, start=True, stop=True)
                for c in range(DC):
                    dst = dst_of(c, t0)
                    if inplace_src:
                        nc.vector.tensor_tensor(srcf[c], srcf[c], mb,
                                                ALU.add)
                        nc.vector.tensor_tensor(dst, srcf[c], ib, ALU.mult)
                    else:
                        t1 = work.tile([128, NT], F32, tag="apt",
                                       name="apt", bufs=2)
                        nc.vector.tensor_tensor(t1[:], srcf[c], mb, ALU.add)
                        nc.vector.tensor_tensor(dst, t1[:], ib, ALU.mult)

        # ---------------------------------------------------- main body
        for rep in range(reps):
          for b in range(B):
            ob1 = list(range(NBLK))
            ob2 = list(range(NBLK - 1, -1, -1))

            # ===== N1: stream x in, layernorm -> xh (padded fp8)
            xblk = {}
            for blk in ob1:
                for c in range(DC):
                    t = work.tile([128, NT], F32R, tag=f"xb{c}",
                                  name=f"xb{c}", bufs=3)
                    nc.sync.dma_start(
                        t[:], xT_d[b, c * 128:(c + 1) * 128,
                                   blk * NT:(blk + 1) * NT].bitcast(F32R))
                    xblk[(blk, c)] = t

            norm(lambda c, t0: xblk[(t0 // NT, c)][:],
                 lambda c, t0: xh_t[c][:, NB + t0:NB + t0 + NT],
                 ob1, inplace_src=False, src_is_r=True)

            # column-shifted planes (SBUF->SBUF DMA), after all applies
            for c in range(DC):
                src = xh_t[c][:, 0:PADL].rearrange("p (r cc) -> p r cc",
                                                   cc=NB)
                xm3 = xs_t[c][:, 0:PADL].rearrange("p (r cc) -> p r cc",
                                                   cc=NB)
                xp3 = xs_t[c][:, PADL:2 * PADL].rearrange(
                    "p (r cc) -> p r cc", cc=NB)
                nc.sync.dma_start(xm3[:, :, 1:NB], src[:, :, 0:NB - 1])
                nc.sync.dma_start(xp3[:, :, 0:NB - 1], src[:, :, 1:NB])

            # ===== conv: depthwise 3x3, 4 DR pairs + 1 single per slab
            for c in range(DC):
                dg = diag_t[c]
                xh = xh_t[c][:]
                xm = xs_t[c][:, 0:PADL]
                xp = xs_t[c][:, PADL:2 * PADL]
                for slab in range(NBLK):
                    r0 = 7 * slab
                    cp = hp_tile("conv_ps")[:, 0:NT]
                    nc.tensor.matmul(
                        cp, dg[:, 1024:1152],
                        xh_t[c][:, (r0 + 1) * NB:(r0 + 1) * NB + NT],
                        start=True, stop=False)
                    for pi, plane in ((0, xm), (1, xh), (2, xp)):
                        lhs = dg[:, pi * 256:(pi + 1) * 256].rearrange(
                            "p (s m) -> p s m", s=2)
                        nc.tensor.matmul(
                            cp, lhs, _pair_ap(plane, r0 * NB, 2 * NB, NT),
                            start=False, stop=False, perf_mode=DRM)
                    lhs = dg[:, 768:1024].rearrange("p (s m) -> p s m", s=2)
                    nc.tensor.matmul(
                        cp, lhs,
                        _pair_ap(xs_t[c][:], (r0 + 1) * NB, PADL, NT),
                        start=False, stop=True, perf_mode=DRM)
                    if c < 2:
                        dst = hc01[:, c * L + slab * NT:
                                   c * L + (slab + 1) * NT]
                    else:
                        dst = hc2o[:, slab * NT:(slab + 1) * NT]
                    nc.scalar.activation(dst, cp, AF.Identity,
                                         bias=dwb_t[:, c:c + 1],
                                         scale=1.0 / SC)

            # ===== bi-minGRU
            hc01_3 = hc01[:].rearrange("p (s l) -> p s l", s=2)
            hc2o_3 = hc2o[:].rearrange("p (s l) -> p s l", s=2)

            def emit_hg(g, q, j, hs_q):
                hp = hp_tile(f"hgh{g}{q}{j}")
                gp = gp_tile(f"hgg{g}{q}{j}")
                wp0 = whg_t[g][:, 0:3072].rearrange("p (s m) -> p s m", s=2)
                wp1 = whg_t[g][:, 3072:6144].rearrange("p (s m) -> p s m",
                                                       s=2)
                for nb2 in range(2):
                    t0 = q * QT + nb2 * NT
                    mv0 = hc01_3[:, :, t0:t0 + NT]
                    mv1 = hc2o_3[:, :, t0:t0 + NT]
                    for moff, ps in ((j * 128, hp), (DI + j * 128, gp)):
                        # pair1 sub1 is a x0.125 ones-channel on the hidden
                        # path (adds 0.5*SW to hp) and zeros on the gate path
                        out = ps[:, nb2 * 512:nb2 * 512 + NT]
                        nc.tensor.matmul(out, wp0[:, :, moff:moff + 128],
                                         mv0, start=True, stop=False,
                                         perf_mode=DRM)
                        nc.tensor.matmul(out, wp1[:, :, moff:moff + 128],
                                         mv1, start=False, stop=True,
                                         perf_mode=DRM)
                # gate math (bf16)
                s = work.tile([128, QT], BF16, tag="s", name="s", bufs=2)
                a = work.tile([128, QT], BF16, tag="a", name="a", bufs=2)
                st = work.tile([128, QT], BF16, tag="st", name="st", bufs=2)
                bb = work.tile([128, QT], BF16, tag="bb", name="bb", bufs=2)
                s2 = s[:].rearrange("p (v h) -> p v h", v=2)
                a2 = a[:].rearrange("p (v h) -> p v h", v=2)
                st2 = st[:].rearrange("p (v h) -> p v h", v=2)
                nc.scalar.activation(s2, pair2(hp), AF.Sigmoid,
                                     scale=1.0 / SW, bias=neghalf[:])
                nc.scalar.activation(a2, pair2(gp), AF.Sigmoid,
                                     scale=-1.0 / SW)
                # st = max(h + 0.5, sigmoid(h));  b = (1-a)*st = st - a*st
                nc.vector.scalar_tensor_tensor(st2, pair2(hp), 1.0 / SW,
                                               s2, ALU.mult, ALU.max)
                nc.gpsimd.tensor_tensor(bb[:], a[:], st[:], ALU.mult)
                nc.vector.tensor_tensor(bb[:], st[:], bb[:], ALU.subtract)
                # scan: h = a*h + b
                hs_out = hs_q[j // 2][:, (j % 2) * QT:(j % 2 + 1) * QT]
                first = (q == 0) if g == 0 else (q == NQ - 1)
                init = 0.0 if first else carry[g][:, j:j + 1]
                if g == 0:
                    nc.vector.tensor_tensor_scan(hs_out, a[:], bb[:], init,
                                                 ALU.mult, ALU.add)
                else:
                    rv = slice(None, None, -1)
                    nc.vector.tensor_tensor_scan(hs_out[:, rv], a[:, rv],
                                                 bb[:, rv], init,
                                                 ALU.mult, ALU.add)
                col = (QT - 1) if g == 0 else 0
                nc.gpsimd.tensor_copy(carry[g][:, j:j + 1],
                                      hs_out[:, col:col + 1])

            def emit_wout(g, q, hs_q):
                # bf16 matmuls (hs/wout stay bf16 for accuracy)
                wt = wout_t[g]
                for dc in range(DC):
                    yp = hp_tile(f"y{g}{q}{dc}")
                    for nb2 in range(2):
                        out = yp[:, nb2 * 512:nb2 * 512 + NT]
                        for k in range(6):
                            lhs = wt[:, k * 384 + dc * 128:
                                     k * 384 + dc * 128 + 128]
                            mv = hs_q[k // 2][:, (k % 2) * QT + nb2 * NT:
                                              (k % 2) * QT + (nb2 + 1) * NT]
                            nc.tensor.matmul(out, lhs, mv, start=(k == 0),
                                             stop=(k == 5))
                    for nb2 in range(2):
                        t0 = q * QT + nb2 * NT
                        ysl = y_t[dc][:, t0:t0 + NT]
                        ps = yp[:, nb2 * 512:nb2 * 512 + NT]
                        if g == 0:
                            xr = work.tile([128, NT], F32, tag="xr",
                                           name="xr", bufs=3)
                            nc.sync.dma_start(
                                xr[:], xT_d[b, dc * 128:(dc + 1) * 128,
                                            t0:t0 + NT])
                            nc.vector.tensor_tensor(ysl, ps, xr[:], ALU.add)
                        else:
                            nc.vector.tensor_tensor(ysl, ps, ysl, ALU.add)

            for g in (0, 1):
                qorder = list(range(NQ)) if g == 0 else \
                    list(range(NQ - 1, -1, -1))
                hs_of = {}
                for qi, q in enumerate(qorder):
                    hs_of[q] = [work.tile([128, 2 * QT], BF16, tag=f"hs{p}",
                                          name=f"hs{p}", bufs=2)
                                for p in range(3)]
                    for j in range(6):
                        emit_hg(g, q, j, hs_of[q])
                    if qi > 0:
                        emit_wout(g, qorder[qi - 1], hs_of[qorder[qi - 1]])
                emit_wout(g, qorder[NQ - 1], hs_of[qorder[NQ - 1]])

            # ===== N2
            def yh_dst(c, t0):
                if c < 2:
                    return yh01[:, c * L + t0:c * L + t0 + NT]
                return yh2o[:, t0:t0 + NT]

            norm(lambda c, t0: y_t[c][:, t0:t0 + NT], yh_dst,
                 ob2, inplace_src=False, src_is_r=False)

            # ===== MLP
            yh01_3 = yh01[:].rearrange("p (s l) -> p s l", s=2)
            yh2o_3 = yh2o[:].rearrange("p (s l) -> p s l", s=2)
            p1dr = p1_t[:, 0:3072].rearrange("p (s m) -> p s m", s=2)
            p1p1 = p1_t[:, 3072:6144].rearrange("p (s m) -> p s m", s=2)
            for blk in ob2:
                t0 = blk * NT
                mv_dr = yh01_3[:, :, t0:t0 + NT]
                mv_sg = yh2o_3[:, :, t0:t0 + NT]
                qts = [work.tile([128, 2 * NT], F8, tag=f"qt{p}",
                                 name=f"qt{p}", bufs=1) for p in range(6)]
                for p in range(6):
                    qp = hp_tile(f"q{blk}{p}")
                    for half in range(2):
                        mc = p * 2 + half
                        out = qp[:, half * 512:half * 512 + NT]
                        nc.tensor.matmul(
                            out, p1dr[:, :, mc * 128:(mc + 1) * 128],
                            mv_dr, start=True, stop=False, perf_mode=DRM)
                        nc.tensor.matmul(
                            out, p1p1[:, :, mc * 128:(mc + 1) * 128],
                            mv_sg, start=False, stop=True, perf_mode=DRM)
                        nc.scalar.activation(
                            qts[p][:, half * NT:(half + 1) * NT],
                            qp[:, half * 512:half * 512 + NT], AF.Gelu,
                            bias=p1b_t[:, mc:mc + 1], scale=1.0 / SW)
                for dc in range(DC):
                    op = gp_tile(f"o{blk}{dc}")[:, 0:NT]
                    for p in range(6):
                        lhs = p2_t[:, p * 768:(p + 1) * 768].rearrange(
                            "p (s m) -> p s m",
                            s=2)[:, :, dc * 128:dc * 128 + 128]
                        mv = qts[p][:].rearrange("p (s h) -> p s h", s=2)
                        nc.tensor.matmul(op, lhs, mv, start=(p == 0),
                                         stop=(p == 5), perf_mode=DRM)
                    ot = work.tile([128, NT], F32, tag="ot", name="ot",
                                   bufs=3)
                    nc.scalar.activation(ot[:], op, AF.Identity,
                                         bias=p2b_t[:, dc:dc + 1],
                                         scale=1.0 / SW)
                    nc.gpsimd.tensor_tensor(ot[:], ot[:],
                                            y_t[dc][:, t0:t0 + NT], ALU.add)
                    nc.sync.dma_start(
                        out_d[b, dc * 128:(dc + 1) * 128, t0:t0 + NT],
                        ot[:])

    return nc


# ---------------------------------------------------------------- host side
_NC_CACHE = {}


def _get_nc():
    key = "fp8"
    if key not in _NC_CACHE:
        nc = build_kernel()
        _fix_multiwaits(nc)
        _NC_CACHE[key] = nc
    return _NC_CACHE[key]


def _to_f8(a):
    return np.asarray(a, np.float32).astype(ml_dtypes.float8_e4m3)


def _prep_weights(inp):
    f = np.float32
    dw_w = np.asarray(inp["dw_w"], f)          # [D,1,3,3]
    norm_w = np.asarray(inp["norm_w"], f)
    norm_b = np.asarray(inp["norm_b"], f)
    dw_wf = dw_w[:, 0] * norm_w[:, None, None]     # [D,3,3]
    dw_bf = np.asarray(inp["dw_b"], f) + norm_b * dw_w[:, 0].sum(axis=(1, 2))
    p1_w = np.asarray(inp["p1_w"], f)
    p1f = p1_w * np.asarray(inp["norm2_w"], f)[:, None]
    p1bf = np.asarray(inp["p1_b"], f) + np.asarray(inp["norm2_b"], f) @ p1_w

    # conv diag blocks, fp8, x8: [DC, 128, 1152]
    # 4 DR pairs (256 cols each: sub0|sub1) + center single (128):
    #   pair0: (-1,-1)&(+1,-1) on xm; pair1: (-1,0)&(+1,0) on xh;
    #   pair2: (-1,+1)&(+1,+1) on xp; pair3: (0,-1)&(0,+1) on xm|xp
    diag = np.zeros((DC, 128, 1152), f)
    ar = np.arange(128)
    pairs = [((0, 0), (2, 0)), ((0, 1), (2, 1)), ((0, 2), (2, 2)),
             ((1, 0), (1, 2))]
    for c in range(DC):
        wch = dw_wf[c * 128:(c + 1) * 128] * SC      # [128, 3, 3]
        for pi, (ta, tb) in enumerate(pairs):
            diag[c, ar, pi * 256 + ar] = wch[:, ta[0], ta[1]]
            diag[c, ar, pi * 256 + 128 + ar] = wch[:, tb[0], tb[1]]
        diag[c, ar, 1024 + ar] = wch[:, 1, 1]

    def pack_whg(w, hidden_bias):   # [384, M] -> [128, 4M]: 2 DR pairs;
        # pair1 sub1 = ones-channel weights (0.5*SW/128 on hidden cols)
        ws = np.asarray(w, f) * SW
        M = ws.shape[1]
        p0 = np.stack([ws[0:128], ws[128:256]], axis=1).reshape(128, 2 * M)
        sub1 = np.zeros((128, M), f)
        if hidden_bias:
            sub1[:, 0:DI] = 0.5 * SW / 128.0
        pr1 = np.stack([ws[256:384], sub1], axis=1).reshape(128, 2 * M)
        return np.concatenate([p0, pr1], axis=1)

    def pack_kx(w):    # [256*np, 384] -> [128, np*768], all DR pairs
        ws = np.asarray(w, f) * SW
        outp = []
        for p in range(ws.shape[0] // 256):
            blk = ws[p * 256:(p + 1) * 256]
            outp.append(np.stack([blk[0:128], blk[128:256]],
                                 axis=1).reshape(128, 768))
        return np.concatenate(outp, axis=1)

    def pack_bf(w):    # [K, 384] -> [128, (K//128)*384] bf16, unscaled
        ws = np.asarray(w, f)
        outp = [ws[k * 128:(k + 1) * 128] for k in range(ws.shape[0] // 128)]
        return np.concatenate(outp, axis=1).astype(ml_dtypes.bfloat16)

    return dict(
        whg0=_to_f8(pack_whg(inp["gru1_whg"], True)),
        whg1=_to_f8(pack_whg(inp["gru2_whg"], True)),
        wout0=pack_bf(inp["gru1_wout"]),
        wout1=pack_bf(inp["gru2_wout"]),
        p1=_to_f8(pack_whg(p1f, False)),
        p2=_to_f8(pack_kx(inp["p2_w"])),
        diag=_to_f8(diag),
        dwb=np.ascontiguousarray(dw_bf.reshape(DC, 128).T, f),
        p1b=np.ascontiguousarray(p1bf.reshape(MLPC, 128).T, f),
        p2b=np.ascontiguousarray(
            np.asarray(inp["p2_b"], f).reshape(DC, 128).T, f),
    )


def kernel(**inputs):
    x = np.asarray(inputs["x"], np.float32)    # [16, L, D]
    w = _prep_weights(inputs)
    nc = _get_nc()

    in_maps = []
    for core in range(NCORES):
        xb = x[core * B:(core + 1) * B]                   # [B, L, D]
        xT = np.ascontiguousarray(xb.transpose(0, 2, 1))  # [B, D, L]
        m = dict(w)
        m["xT"] = xT
        in_maps.append(m)

    res = run_bass_kernel_spmd(nc, in_maps, core_ids=list(range(NCORES)))
    outs = []
    for core in range(NCORES):
        oT = res.results[core]["outT"]                    # [B, D, L]
        outs.append(oT.transpose(0, 2, 1))                # [B, L, D]
    return np.ascontiguousarray(np.concatenate(outs, axis=0), np.float32)


# revision 23
# speedup vs baseline: 1.1023x; 1.1023x over previous
"""Trainium2 Bass kernel for nn_Block2DGRU (norm->dwconv3x3->bi-minGRU->norm->MLP).

fp8(e4m3)+DoubleRow matmuls for conv/GRU/MLP (weights pre-scaled x32, conv x8,
unscaled via activation `scale=`); f32r (bitcast) for LN stats/broadcasts;
bf16 gate math on DVE (2x mode); the `h+0.5` pass on the Pool engine; conv
uses zero-padded image rows + SBUF->SBUF DMA column shifts, with the 9 taps
packed as 4 DoubleRow pairs + 1 single matmul.  SPMD over 8 NeuronCores,
2 batch elements per core.

Device layout: [feature_on_partitions, time_on_free].  minGRU recurrence
h_t = a_t*h_{t-1} + b_t runs on the DVE via tensor_tensor_scan; dir-2 is the
same scan with reversed access patterns.
"""
import numpy as np
import ml_dtypes

import concourse.bass as bass
import concourse.tile as tile
import concourse.mybir as mybir
from concourse.bass_utils import run_bass_kernel_spmd

F32 = mybir.dt.float32
F32R = mybir.dt.float32r
BF16 = mybir.dt.bfloat16
F8 = mybir.dt.float8e4
AF = mybir.ActivationFunctionType
ALU = mybir.AluOpType
DRM = mybir.MatmulPerfMode.DoubleRow

# dims
NB = 56
L = NB * NB            # 3136
D = 384                # dim
DC = 3                 # dim chunks of 128
DI = 768               # gru inner
MLPC = 12
B = 2                  # batch per core
NCORES = 8
NT = 392               # time block (= 7 image rows)
NBLK = L // NT         # 8
QT = 784               # scan quarter (= 2 blocks)
NQ = L // QT           # 4
EPS = 1e-5
PADL = 58 * NB         # image rows -1..56 -> 3248
SW = 32.0              # prescale for gru/mlp weights
SC = 8.0               # prescale for conv taps

MM_DT = F32R           # kept for test.py compat


def _pair_ap(flat_ap, off, stride, nt, nb=NB):
    """[128, 2, nt] read AP over a flat [128, X] AP: subtiles at element
    offsets `off` and `off+stride` (stride % nb == 0), each nt wide."""
    a3 = flat_ap.rearrange("p (r c) -> p r c", c=nb)
    s = stride // nb
    r0 = off // nb
    sl = a3[:, r0:r0 + s + 1:s, :]
    v = sl.ap
    v[2] = (1, nt)
    out = sl.copy()
    out.ap = v
    return out


# ---------------------------------------------------------------- wait fix
def _fix_multiwaits(nc):
    """The walrus accepts at most ONE sync wait per instruction; hoist
    extras into wait-only NoOps on the same engine (streams are in-order)."""
    n = 0
    cnt = [0]
    for f in nc.m.functions:
        for bb in f.blocks:
            out = []
            for inst in bb.instructions:
                si = inst.sync_info
                if si is not None and si.on_wait is not None and len(si.on_wait) > 1:
                    waits = list(si.on_wait)
                    for w in waits[:-1]:
                        cnt[0] += 1
                        nop = mybir.InstNoOp(
                            name=f"I-waitfix-{cnt[0]}",
                            sync_info=mybir.SyncInfo(on_wait=[w], on_update=[]),
                        )
                        nop.engine = inst.engine
                        out.append(nop)
                    inst.sync_info = mybir.SyncInfo(
                        on_wait=[waits[-1]], on_update=list(si.on_update or [])
                    )
                    n += 1
                out.append(inst)
            bb.instructions = out
    return n


# ---------------------------------------------------------------- builder
def build_kernel(reps=1):
    nc = bass.Bass("TRN2", target_bir_lowering=False, debug=False,
                   num_devices=NCORES)

    xT_d = nc.dram_tensor("xT", [B, D, L], F32, kind="ExternalInput").ap()
    whg_d = [nc.dram_tensor(f"whg{g}", [128, 6144], F8,
                            kind="ExternalInput").ap() for g in range(2)]
    wout_d = [nc.dram_tensor(f"wout{g}", [128, 2304], BF16,
                             kind="ExternalInput").ap() for g in range(2)]
    p1_d = nc.dram_tensor("p1", [128, 6144], F8, kind="ExternalInput").ap()
    p2_d = nc.dram_tensor("p2", [128, 4608], F8, kind="ExternalInput").ap()
    diag_d = nc.dram_tensor("diag", [DC, 128, 1152], F8,
                            kind="ExternalInput").ap()
    dwb_d = nc.dram_tensor("dwb", [128, DC], F32, kind="ExternalInput").ap()
    p1b_d = nc.dram_tensor("p1b", [128, MLPC], F32, kind="ExternalInput").ap()
    p2b_d = nc.dram_tensor("p2b", [128, DC], F32, kind="ExternalInput").ap()
    out_d = nc.dram_tensor("outT", [B, D, L], F32, kind="ExternalOutput").ap()

    from contextlib import ExitStack
    with tile.TileContext(nc) as tc, ExitStack() as ctx:
        big = ctx.enter_context(tc.tile_pool(name="big", bufs=1))
        wpool = ctx.enter_context(tc.tile_pool(name="wpool", bufs=1))
        work = ctx.enter_context(tc.tile_pool(name="work", bufs=2))
        psum = ctx.enter_context(tc.tile_pool(name="psum", bufs=2,
                                              space="PSUM"))

        # ---- persistent constants / weights
        ones_col = wpool.tile([128, 1], F32, tag="ones_col", name="ones_col")
        nc.vector.memset(ones_col[:], 1.0)
        ones_colr = wpool.tile([128, 1], F32R, tag="ones_colr",
                               name="ones_colr")
        nc.scalar.activation(ones_colr[:], ones_col[:], AF.Copy)
        ones1 = wpool.tile([1, 128], BF16, tag="ones1", name="ones1")
        nc.vector.memset(ones1[:], 1.0)
        ones_colb = wpool.tile([128, 1], BF16, tag="ones_colb",
                               name="ones_colb")
        nc.vector.memset(ones_colb[:], 1.0)
        dwb_t = wpool.tile([128, DC], F32, tag="dwb", name="dwb")
        nc.sync.dma_start(dwb_t[:], dwb_d)
        p1b_t = wpool.tile([128, MLPC], F32, tag="p1b", name="p1b")
        nc.sync.dma_start(p1b_t[:], p1b_d)
        p2b_t = wpool.tile([128, DC], F32, tag="p2b", name="p2b")
        nc.sync.dma_start(p2b_t[:], p2b_d)
        eps_t = wpool.tile([1, 1], F32, tag="eps", name="eps")
        nc.vector.memset(eps_t[:], EPS)
        neghalf = wpool.tile([128, 1], F32, tag="neghalf", name="neghalf")
        nc.vector.memset(neghalf[:], -0.5)

        whg_t, wout_t = [], []
        for g in range(2):
            t = wpool.tile([128, 6144], F8, tag=f"whg{g}", name=f"whg{g}")
            nc.sync.dma_start(t[:], whg_d[g])
            whg_t.append(t)
            t = wpool.tile([128, 2304], BF16, tag=f"wout{g}", name=f"wout{g}")
            nc.sync.dma_start(t[:], wout_d[g])
            wout_t.append(t)
        p1_t = wpool.tile([128, 6144], F8, tag="p1w", name="p1w")
        nc.sync.dma_start(p1_t[:], p1_d)
        p2_t = wpool.tile([128, 4608], F8, tag="p2w", name="p2w")
        nc.sync.dma_start(p2_t[:], p2_d)
        diag_t = []
        for c in range(DC):
            t = wpool.tile([128, 1152], F8, tag=f"diag{c}", name=f"diag{c}")
            nc.sync.dma_start(t[:], diag_d[c])
            diag_t.append(t)

        # ---- persistent big buffers
        xh_t = [big.tile([128, PADL], F8, tag=f"xh{c}", name=f"xh{c}")
                for c in range(DC)]
        xs_t = [big.tile([128, 2 * PADL], F8, tag=f"xs{c}", name=f"xs{c}")
                for c in range(DC)]
        hc01 = big.tile([128, 2 * L], F8, tag="hc01", name="hc01")
        hc2o = big.tile([128, 2 * L], F8, tag="hc2o", name="hc2o")
        y_t = [big.tile([128, L], F32, tag=f"y{c}", name=f"y{c}")
               for c in range(DC)]
        yh01 = big.tile([128, 2 * L], F8, tag="yh01", name="yh01")
        yh2o = big.tile([128, 2 * L], F8, tag="yh2o", name="yh2o")
        carry = [big.tile([128, 6], F32, tag=f"carry{g}", name=f"carry{g}")
                 for g in range(2)]

        # second halves of hc2o/yh2o hold a constant ones channel-block
        nc.gpsimd.memset(hc2o[:, L:2 * L], 1.0)
        nc.gpsimd.memset(yh2o[:, L:2 * L], 1.0)

        # one-time zeroing: pad rows (image rows -1, 56) + shift edge cols
        for c in range(DC):
            nc.gpsimd.memset(xh_t[c][:, 0:NB], 0.0)
            nc.gpsimd.memset(xh_t[c][:, PADL - NB:PADL], 0.0)
            xm3 = xs_t[c][:, 0:PADL].rearrange("p (r cc) -> p r cc", cc=NB)
            nc.gpsimd.memset(xm3[:, :, 0:1], 0.0)
            xp3 = xs_t[c][:, PADL:2 * PADL].rearrange("p (r cc) -> p r cc",
                                                      cc=NB)
            nc.gpsimd.memset(xp3[:, :, NB - 1:NB], 0.0)

        def hp_tile(name):
            return psum.tile([128, 1024], F32, tag="hp", name=name)

        def gp_tile(name):
            return psum.tile([128, 1024], F32, tag="gp", name=name)

        def pair2(t):
            """[128,1024] psum tile -> [128,2,NT] view (cols 0.., 512..)."""
            return t[:].rearrange("p (s h) -> p s h", s=2)[:, :, 0:NT]

        # ---------------------------------------------------- layernorm
        def norm(src_of, dst_of, order, inplace_src, src_is_r):
            """LN over chunks; src_of(c, t0) -> [128,NT] AP (dtype F32R
            when src_is_r else F32); dst_of(c, t0) -> fp8 dst AP.
            inplace_src: srcs may be clobbered by the apply."""
            for blk in order:
                t0 = blk * NT
                srcs = [src_of(c, t0) for c in range(DC)]
                srcf = [(sc.bitcast(F32) if src_is_r else sc) for sc in srcs]
                rows = work.tile([1, 2 * NT], BF16, tag="rows", name="rows",
                                 bufs=2)
                sq = [work.tile([128, NT], BF16, tag=f"sq{c}", name=f"sq{c}",
                                bufs=2) for c in range(DC)]
                sp = hp_tile("stat_ps")
                s_ps = sp[0:1, 0:NT]
                q_ps = sp[0:1, 512:512 + NT]
                for c in range(DC):
                    nc.gpsimd.tensor_tensor(sq[c][:], srcf[c], srcf[c],
                                            ALU.mult)
                for c in range(DC):
                    # f32r when the producer is a DMA (rounding-exempt);
                    # plain f32 (4 cyc) when the producer is the DVE
                    if src_is_r:
                        nc.tensor.matmul(s_ps, ones_colr[:], srcs[c],
                                         start=(c == 0), stop=(c == DC - 1))
                    else:
                        nc.tensor.matmul(s_ps, ones_col[:], srcs[c],
                                         start=(c == 0), stop=(c == DC - 1))
                for c in range(DC):
                    nc.tensor.matmul(q_ps, ones_colb[:], sq[c][:],
                                     start=(c == 0), stop=(c == DC - 1))
                numu = rows[0:1, 0:NT]
                inv = rows[0:1, NT:2 * NT]
                nc.scalar.activation(numu, s_ps, AF.Copy, scale=-1.0 / D)
                sf = work.tile([1, 4 * NT], F32, tag="statf", name="statf",
                               bufs=2)
                m2 = sf[0:1, 0:NT]
                ve = sf[0:1, NT:2 * NT]
                sd = sf[0:1, 2 * NT:3 * NT]
                nc.scalar.activation(m2, s_ps, AF.Square, scale=1.0 / D)
                nc.vector.scalar_tensor_tensor(ve, q_ps, 1.0 / D, m2,
                                               ALU.mult, ALU.subtract)
                # inv = 1/sqrt(ve+eps) = exp(-0.5*ln(ve+eps)); table ops
                # stay on Act (DVE reciprocal is a ~2.6us multi-pass op)
                nc.scalar.activation(sd, ve, AF.Ln, bias=eps_t[:])
                nc.scalar.activation(inv, sd, AF.Exp, scale=-0.5)
                # broadcast -mu / inv to 128 partitions (K=1 bf16 matmuls)
                bc = gp_tile("bc_ps")
                mb = bc[:, 0:NT]
                ib = bc[:, 512:512 + NT]
                nc.tensor.matmul(mb, ones1[:], numu, start=True, stop=True)
                nc.tensor.matmul(ib, ones1[:], inv, start=True, stop=True)
                for c in range(DC):
                    dst = dst_of(c, t0)
                    if inplace_src:
                        nc.vector.tensor_tensor(srcf[c], srcf[c], mb,
                                                ALU.add)
                        nc.vector.tensor_tensor(dst, srcf[c], ib, ALU.mult)
                    else:
                        t1 = work.tile([128, NT], F32, tag="apt",
                                       name="apt", bufs=2)
                        nc.vector.tensor_tensor(t1[:], srcf[c], mb, ALU.add)
                        nc.vector.tensor_tensor(dst, t1[:], ib, ALU.mult)

        # ---------------------------------------------------- main body
        for rep in range(reps):
          for b in range(B):
            ob1 = list(range(NBLK))
            ob2 = list(range(NBLK - 1, -1, -1))

            # ===== N1: stream x in, layernorm -> xh (padded fp8)
            xblk = {}
            for blk in ob1:
                for c in range(DC):
                    t = work.tile([128, NT], F32R, tag=f"xb{c}",
                                  name=f"xb{c}", bufs=3)
                    nc.sync.dma_start(
                        t[:], xT_d[b, c * 128:(c + 1) * 128,
                                   blk * NT:(blk + 1) * NT].bitcast(F32R))
                    xblk[(blk, c)] = t

            norm(lambda c, t0: xblk[(t0 // NT, c)][:],
                 lambda c, t0: xh_t[c][:, NB + t0:NB + t0 + NT],
                 ob1, inplace_src=False, src_is_r=True)

            # column-shifted planes (SBUF->SBUF DMA), after all applies
            for c in range(DC):
                src = xh_t[c][:, 0:PADL].rearrange("p (r cc) -> p r cc",
                                                   cc=NB)
                xm3 = xs_t[c][:, 0:PADL].rearrange("p (r cc) -> p r cc",
                                                   cc=NB)
                xp3 = xs_t[c][:, PADL:2 * PADL].rearrange(
                    "p (r cc) -> p r cc", cc=NB)
                nc.sync.dma_start(xm3[:, :, 1:NB], src[:, :, 0:NB - 1])
                nc.sync.dma_start(xp3[:, :, 0:NB - 1], src[:, :, 1:NB])

            # ===== conv: depthwise 3x3, 4 DR pairs + 1 single per slab
            for c in range(DC):
                dg = diag_t[c]
                xh = xh_t[c][:]
                xm = xs_t[c][:, 0:PADL]
                xp = xs_t[c][:, PADL:2 * PADL]
                for slab in range(NBLK):
                    r0 = 7 * slab
                    cp = hp_tile("conv_ps")[:, 0:NT]
                    nc.tensor.matmul(
                        cp, dg[:, 1024:1152],
                        xh_t[c][:, (r0 + 1) * NB:(r0 + 1) * NB + NT],
                        start=True, stop=False)
                    for pi, plane in ((0, xm), (1, xh), (2, xp)):
                        lhs = dg[:, pi * 256:(pi + 1) * 256].rearrange(
                            "p (s m) -> p s m", s=2)
                        nc.tensor.matmul(
                            cp, lhs, _pair_ap(plane, r0 * NB, 2 * NB, NT),
                            start=False, stop=False, perf_mode=DRM)
                    lhs = dg[:, 768:1024].rearrange("p (s m) -> p s m", s=2)
                    nc.tensor.matmul(
                        cp, lhs,
                        _pair_ap(xs_t[c][:], (r0 + 1) * NB, PADL, NT),
                        start=False, stop=True, perf_mode=DRM)
                    if c < 2:
                        dst = hc01[:, c * L + slab * NT:
                                   c * L + (slab + 1) * NT]
                    else:
                        dst = hc2o[:, slab * NT:(slab + 1) * NT]
                    nc.scalar.activation(dst, cp, AF.Identity,
                                         bias=dwb_t[:, c:c + 1],
                                         scale=1.0 / SC)

            # ===== bi-minGRU
            hc01_3 = hc01[:].rearrange("p (s l) -> p s l", s=2)
            hc2o_3 = hc2o[:].rearrange("p (s l) -> p s l", s=2)

            def emit_hg(g, q, j, hs_q):
                hp = hp_tile(f"hgh{g}{q}{j}")
                gp = gp_tile(f"hgg{g}{q}{j}")
                wp0 = whg_t[g][:, 0:3072].rearrange("p (s m) -> p s m", s=2)
                wp1 = whg_t[g][:, 3072:6144].rearrange("p (s m) -> p s m",
                                                       s=2)
                for nb2 in range(2):
                    t0 = q * QT + nb2 * NT
                    mv0 = hc01_3[:, :, t0:t0 + NT]
                    mv1 = hc2o_3[:, :, t0:t0 + NT]
                    for moff, ps in ((j * 128, hp), (DI + j * 128, gp)):
                        # pair1 sub1 is a x0.125 ones-channel on the hidden
                        # path (adds 0.5*SW to hp) and zeros on the gate path
                        out = ps[:, nb2 * 512:nb2 * 512 + NT]
                        nc.tensor.matmul(out, wp0[:, :, moff:moff + 128],
                                         mv0, start=True, stop=False,
                                         perf_mode=DRM)
                        nc.tensor.matmul(out, wp1[:, :, moff:moff + 128],
                                         mv1, start=False, stop=True,
                                         perf_mode=DRM)
                # gate math (bf16)
                s = work.tile([128, QT], BF16, tag="s", name="s", bufs=2)
                a = work.tile([128, QT], BF16, tag="a", name="a", bufs=2)
                st = work.tile([128, QT], BF16, tag="st", name="st", bufs=2)
                bb = work.tile([128, QT], BF16, tag="bb", name="bb", bufs=2)
                s2 = s[:].rearrange("p (v h) -> p v h", v=2)
                a2 = a[:].rearrange("p (v h) -> p v h", v=2)
                st2 = st[:].rearrange("p (v h) -> p v h", v=2)
                nc.scalar.activation(s2, pair2(hp), AF.Sigmoid,
                                     scale=1.0 / SW, bias=neghalf[:])
                nc.scalar.activation(a2, pair2(gp), AF.Sigmoid,
                                     scale=-1.0 / SW)
                # st = max(h + 0.5, sigmoid(h));  b = (1-a)*st = st - a*st
                nc.vector.scalar_tensor_tensor(st2, pair2(hp), 1.0 / SW,
                                               s2, ALU.mult, ALU.max)
                nc.vector.tensor_tensor(bb[:], a[:], st[:], ALU.mult)
                nc.vector.tensor_tensor(bb[:], st[:], bb[:], ALU.subtract)
                # scan: h = a*h + b
                hs_out = hs_q[j // 2][:, (j % 2) * QT:(j % 2 + 1) * QT]
                first = (q == 0) if g == 0 else (q == NQ - 1)
                init = 0.0 if first else carry[g][:, j:j + 1]
                if g == 0:
                    nc.vector.tensor_tensor_scan(hs_out, a[:], bb[:], init,
                                                 ALU.mult, ALU.add)
                else:
                    rv = slice(None, None, -1)
                    nc.vector.tensor_tensor_scan(hs_out[:, rv], a[:, rv],
                                                 bb[:, rv], init,
                                                 ALU.mult, ALU.add)
                col = (QT - 1) if g == 0 else 0
                nc.gpsimd.tensor_copy(carry[g][:, j:j + 1],
                                      hs_out[:, col:col + 1])

            def emit_wout(g, q, hs_q):
                # bf16 matmuls (hs/wout stay bf16 for accuracy)
                wt = wout_t[g]
                for dc in range(DC):
                    yp = hp_tile(f"y{g}{q}{dc}")
                    for nb2 in range(2):
                        out = yp[:, nb2 * 512:nb2 * 512 + NT]
                        for k in range(6):
                            lhs = wt[:, k * 384 + dc * 128:
                                     k * 384 + dc * 128 + 128]
                            mv = hs_q[k // 2][:, (k % 2) * QT + nb2 * NT:
                                              (k % 2) * QT + (nb2 + 1) * NT]
                            nc.tensor.matmul(out, lhs, mv, start=(k == 0),
                                             stop=(k == 5))
                    for nb2 in range(2):
                        t0 = q * QT + nb2 * NT
                        ysl = y_t[dc][:, t0:t0 + NT]
                        ps = yp[:, nb2 * 512:nb2 * 512 + NT]
                        if g == 0:
                            xr = work.tile([128, NT], F32, tag="xr",
                                           name="xr", bufs=3)
                            nc.sync.dma_start(
                                xr[:], xT_d[b, dc * 128:(dc + 1) * 128,
                                            t0:t0 + NT])
                            nc.vector.tensor_tensor(ysl, ps, xr[:], ALU.add)
                        else:
                            nc.vector.tensor_tensor(ysl, ps, ysl, ALU.add)

            for g in (0, 1):
                qorder = list(range(NQ)) if g == 0 else \
                    list(range(NQ - 1, -1, -1))
                hs_of = {}
                for qi, q in enumerate(qorder):
                    hs_of[q] = [work.tile([128, 2 * QT], BF16, tag=f"hs{p}",
                                          name=f"hs{p}", bufs=2)
                                for p in range(3)]
                    for j in range(6):
                        emit_hg(g, q, j, hs_of[q])
                    if qi > 0:
                        emit_wout(g, qorder[qi - 1], hs_of[qorder[qi - 1]])
                emit_wout(g, qorder[NQ - 1], hs_of[qorder[NQ - 1]])

            # ===== N2
            def yh_dst(c, t0):
                if c < 2:
                    return yh01[:, c * L + t0:c * L + t0 + NT]
                return yh2o[:, t0:t0 + NT]

            norm(lambda c, t0: y_t[c][:, t0:t0 + NT], yh_dst,
                 ob2, inplace_src=False, src_is_r=False)

            # ===== MLP
            yh01_3 = yh01[:].rearrange("p (s l) -> p s l", s=2)
            yh2o_3 = yh2o[:].rearrange("p (s l) -> p s l", s=2)
            p1dr = p1_t[:, 0:3072].rearrange("p (s m) -> p s m", s=2)
            p1p1 = p1_t[:, 3072:6144].rearrange("p (s m) -> p s m", s=2)
            for blk in ob2:
                t0 = blk * NT
                mv_dr = yh01_3[:, :, t0:t0 + NT]
                mv_sg = yh2o_3[:, :, t0:t0 + NT]
                qts = [work.tile([128, 2 * NT], F8, tag=f"qt{p}",
                                 name=f"qt{p}", bufs=1) for p in range(6)]
                for p in range(6):
                    qp = hp_tile(f"q{blk}{p}")
                    for half in range(2):
                        mc = p * 2 + half
                        out = qp[:, half * 512:half * 512 + NT]
                        nc.tensor.matmul(
                            out, p1dr[:, :, mc * 128:(mc + 1) * 128],
                            mv_dr, start=True, stop=False, perf_mode=DRM)
                        nc.tensor.matmul(
                            out, p1p1[:, :, mc * 128:(mc + 1) * 128],
                            mv_sg, start=False, stop=True, perf_mode=DRM)
                        nc.scalar.activation(
                            qts[p][:, half * NT:(half + 1) * NT],
                            qp[:, half * 512:half * 512 + NT], AF.Gelu,
                            bias=p1b_t[:, mc:mc + 1], scale=1.0 / SW)
                for dc in range(DC):
                    op = gp_tile(f"o{blk}{dc}")[:, 0:NT]
                    for p in range(6):
                        lhs = p2_t[:, p * 768:(p + 1) * 768].rearrange(
                            "p (s m) -> p s m",
                            s=2)[:, :, dc * 128:dc * 128 + 128]
                        mv = qts[p][:].rearrange("p (s h) -> p s h", s=2)
                        nc.tensor.matmul(op, lhs, mv, start=(p == 0),
                                         stop=(p == 5), perf_mode=DRM)
                    ot = work.tile([128, NT], F32, tag="ot", name="ot",
                                   bufs=3)
                    nc.scalar.activation(ot[:], op, AF.Identity,
                                         bias=p2b_t[:, dc:dc + 1],
                                         scale=1.0 / SW)
                    nc.gpsimd.tensor_tensor(ot[:], ot[:],
                                            y_t[dc][:, t0:t0 + NT], ALU.add)
                    nc.sync.dma_start(
                        out_d[b, dc * 128:(dc + 1) * 128, t0:t0 + NT],
                        ot[:])

    return nc


# ---------------------------------------------------------------- host side
_NC_CACHE = {}


def _get_nc():
    key = "fp8"
    if key not in _NC_CACHE:
        nc = build_kernel()
        _fix_multiwaits(nc)
        _NC_CACHE[key] = nc
    return _NC_CACHE[key]


def _to_f8(a):
    return np.asarray(a, np.float32).astype(ml_dtypes.float8_e4m3)


def _prep_weights(inp):
    f = np.float32
    dw_w = np.asarray(inp["dw_w"], f)          # [D,1,3,3]
    norm_w = np.asarray(inp["norm_w"], f)
    norm_b = np.asarray(inp["norm_b"], f)
    dw_wf = dw_w[:, 0] * norm_w[:, None, None]     # [D,3,3]
    dw_bf = np.asarray(inp["dw_b"], f) + norm_b * dw_w[:, 0].sum(axis=(1, 2))
    p1_w = np.asarray(inp["p1_w"], f)
    p1f = p1_w * np.asarray(inp["norm2_w"], f)[:, None]
    p1bf = np.asarray(inp["p1_b"], f) + np.asarray(inp["norm2_b"], f) @ p1_w

    # conv diag blocks, fp8, x8: [DC, 128, 1152]
    # 4 DR pairs (256 cols each: sub0|sub1) + center single (128):
    #   pair0: (-1,-1)&(+1,-1) on xm; pair1: (-1,0)&(+1,0) on xh;
    #   pair2: (-1,+1)&(+1,+1) on xp; pair3: (0,-1)&(0,+1) on xm|xp
    diag = np.zeros((DC, 128, 1152), f)
    ar = np.arange(128)
    pairs = [((0, 0), (2, 0)), ((0, 1), (2, 1)), ((0, 2), (2, 2)),
             ((1, 0), (1, 2))]
    for c in range(DC):
        wch = dw_wf[c * 128:(c + 1) * 128] * SC      # [128, 3, 3]
        for pi, (ta, tb) in enumerate(pairs):
            diag[c, ar, pi * 256 + ar] = wch[:, ta[0], ta[1]]
            diag[c, ar, pi * 256 + 128 + ar] = wch[:, tb[0], tb[1]]
        diag[c, ar, 1024 + ar] = wch[:, 1, 1]

    def pack_whg(w, hidden_bias):   # [384, M] -> [128, 4M]: 2 DR pairs;
        # pair1 sub1 = ones-channel weights (0.5*SW/128 on hidden cols)
        ws = np.asarray(w, f) * SW
        M = ws.shape[1]
        p0 = np.stack([ws[0:128], ws[128:256]], axis=1).reshape(128, 2 * M)
        sub1 = np.zeros((128, M), f)
        if hidden_bias:
            sub1[:, 0:DI] = 0.5 * SW / 128.0
        pr1 = np.stack([ws[256:384], sub1], axis=1).reshape(128, 2 * M)
        return np.concatenate([p0, pr1], axis=1)

    def pack_kx(w):    # [256*np, 384] -> [128, np*768], all DR pairs
        ws = np.asarray(w, f) * SW
        outp = []
        for p in range(ws.shape[0] // 256):
            blk = ws[p * 256:(p + 1) * 256]
            outp.append(np.stack([blk[0:128], blk[128:256]],
                                 axis=1).reshape(128, 768))
        return np.concatenate(outp, axis=1)

    def pack_bf(w):    # [K, 384] -> [128, (K//128)*384] bf16, unscaled
        ws = np.asarray(w, f)
        outp = [ws[k * 128:(k + 1) * 128] for k in range(ws.shape[0] // 128)]
        return np.concatenate(outp, axis=1).astype(ml_dtypes.bfloat16)

    return dict(
        whg0=_to_f8(pack_whg(inp["gru1_whg"], True)),
        whg1=_to_f8(pack_whg(inp["gru2_whg"], True)),
        wout0=pack_bf(inp["gru1_wout"]),
        wout1=pack_bf(inp["gru2_wout"]),
        p1=_to_f8(pack_whg(p1f, False)),
        p2=_to_f8(pack_kx(inp["p2_w"])),
        diag=_to_f8(diag),
        dwb=np.ascontiguousarray(dw_bf.reshape(DC, 128).T, f),
        p1b=np.ascontiguousarray(p1bf.reshape(MLPC, 128).T, f),
        p2b=np.ascontiguousarray(
            np.asarray(inp["p2_b"], f).reshape(DC, 128).T, f),
    )


def kernel(**inputs):
    x = np.asarray(inputs["x"], np.float32)    # [16, L, D]
    w = _prep_weights(inputs)
    nc = _get_nc()

    in_maps = []
    for core in range(NCORES):
        xb = x[core * B:(core + 1) * B]                   # [B, L, D]
        xT = np.ascontiguousarray(xb.transpose(0, 2, 1))  # [B, D, L]
        m = dict(w)
        m["xT"] = xT
        in_maps.append(m)

    res = run_bass_kernel_spmd(nc, in_maps, core_ids=list(range(NCORES)))
    outs = []
    for core in range(NCORES):
        oT = res.results[core]["outT"]                    # [B, D, L]
        outs.append(oT.transpose(0, 2, 1))                # [B, L, D]
    return np.ascontiguousarray(np.concatenate(outs, axis=0), np.float32)
